# revision 1
# baseline (speedup 1.0000x reference)
"""Llama4 MoE experts kernel for 8 TRN2 NeuronCores (expert-parallel).

Full-input contract: kernel(**inputs) takes the unsharded fp32 arrays and
returns the full fp32 output. Internally: one expert per core; hidden is
contracted as lhsT=weight-tile (stationary), rhs=x^T (moving), so both
matmul stages produce transposed outputs and no on-chip transpose is
needed. Compute in bf16 (fp32 PSUM accumulate), SiLU on ScalarE, gate*up
on VectorE, output fp32.

Shapes (hardcoded, per spec):
  hidden_states [8192, 2048] f32, gate_up_proj [8, 2048, 8192] f32,
  down_proj [8, 4096, 2048] f32 -> out [8192, 2048] f32.
"""

import ml_dtypes
import numpy as np

import concourse.bass as bass
import concourse.mybir as mybir
import concourse.tile as tile
from concourse.bass_utils import run_bass_kernel_spmd

BF16 = ml_dtypes.bfloat16
P = 128
E = 8          # experts == cores
T = 1024       # tokens per expert
H = 2048       # hidden
I = 4096       # expert dim
KH = H // P    # 16 k-tiles for MM1
KI = I // P    # 32 k-tiles for MM2
FG = 32        # gate f-tiles (up tiles are FG..2*FG-1)
HT = H // P    # 16 output h-tiles


class _TileContext(tile.TileContext):
    """TileContext whose tail drain splits sem waits across instructions.

    The stock _drain_and_barrier attaches every outstanding semaphore wait
    to one Drain instruction; core_v3 codegen only allows one sync wait per
    non-EventSemaphore instruction, so kernels touching >1 semaphore at the
    tail fail with "Too many sync wait commands". Re-emit the extra waits
    as standalone wait_ge instructions ahead of a clean drain.
    """

    def _drain_and_barrier(self, tick_clock, wait_clock):
        import bass_rust as _br

        nc = self.nc
        drain_inst = nc.sync.drain()
        wait_clock.add_sem_waits(
            drain_inst.ins, _br.ScopedClock({None: tick_clock.global_clock})
        )
        si = drain_inst.ins.sync_info
        waits = list(si.on_wait or []) if si is not None else []
        if len(waits) > 1:
            si.on_wait = [waits[0]]
            by_num = {h.num: h for h in self.sems.allocated().values()}
            for w in waits[1:]:
                nc.sync.wait_ge(by_num[w.id], w.wait_value)
            nc.sync.drain()
        nc.all_engine_barrier()
        assert self.sems is not None
        popped = nc._tile_sem_poison_stack.pop()
        assert popped is self._sem_poison
        nc.clear_and_free_semaphores(list(self.sems.allocated().values()))
        nc.all_engine_barrier()


def _split_excess_waits(bir: bytes) -> bytes:
    """Rewrite BIR so no instruction carries more sem waits than this
    walrus accepts (1 per regular instruction, 2 per EventSemaphore).
    Excess waits become standalone EventSemaphore instructions emitted
    just before the over-subscribed instruction on the same engine, which
    is semantically identical (same-engine queue order)."""
    import json

    m = json.loads(bir)
    ctr = 0
    for func in m["functions"]:
        for bb in func["blocks"]:
            out = []
            for ins in bb["instructions"]:
                si = ins.get("sync_info")
                waits = (si or {}).get("on_wait") or []
                cap = 2 if ins.get("opcode") == "EventSemaphore" else 1
                if len(waits) > cap:
                    keep = waits[len(waits) - cap :]
                    excess = waits[: len(waits) - cap]
                    for w in excess:
                        ctr += 1
                        out.append(
                            {
                                "debug": ins.get("debug"),
                                "engine": ins["engine"],
                                "ins": [],
                                "name": f"{ins['name']}-wsplit{ctr}",
                                "opcode": "EventSemaphore",
                                "outs": [],
                                "sync_info": {"on_update": [], "on_wait": [w]},
                            }
                        )
                    si["on_wait"] = keep
                out.append(ins)
            bb["instructions"] = out
    return json.dumps(m).encode()


def _build_program():
    bf16 = mybir.dt.bfloat16
    f32 = mybir.dt.float32

    nc = bass.Bass()
    xt_d = nc.declare_dram_parameter("xt", [P, KH, T], bf16, isOutput=False)
    w1_d = nc.declare_dram_parameter("w1", [2 * FG, P, KH, P], bf16, isOutput=False)
    w2_d = nc.declare_dram_parameter("w2", [HT, P, KI, P], bf16, isOutput=False)
    out_d = nc.declare_dram_parameter("out", [HT, P, T], f32, isOutput=True)

    with _TileContext(nc) as tc:
        with (
            tc.tile_pool(name="xp", bufs=1) as xp,
            tc.tile_pool(name="wp", bufs=4) as wp,
            tc.tile_pool(name="gp", bufs=1) as gp,
            tc.tile_pool(name="ap", bufs=2) as ap,
            tc.tile_pool(name="op", bufs=2) as op,
            tc.tile_pool(name="ps", bufs=3, space="PSUM") as ps,
        ):
            x_sb = xp.tile([P, KH, T], bf16)
            nc.sync.dma_start(x_sb[:], xt_d[:])
            g_sb = gp.tile([P, KI, T], bf16)  # gated^T, cached whole

            # MM1: psum[f, t] += W1tile^T @ x^T ; SiLU-gate -> gated^T (bf16)
            for fg in range(FG):
                w1g = wp.tile([P, KH, P], mybir.dt.bfloat16, tag="w1")
                nc.sync.dma_start(w1g[:], w1_d[fg])
                w1u = wp.tile([P, KH, P], mybir.dt.bfloat16, tag="w1")
                nc.sync.dma_start(w1u[:], w1_d[fg + FG])
                ps_g = ps.tile([P, T], f32, tag="ps1")
                ps_u = ps.tile([P, T], f32, tag="ps1")
                for half in range(2):
                    sl = slice(half * 512, (half + 1) * 512)
                    for k in range(KH):
                        nc.tensor.matmul(
                            ps_g[:, sl], w1g[:, k], x_sb[:, k, sl],
                            start=(k == 0), stop=(k == KH - 1),
                        )
                for half in range(2):
                    sl = slice(half * 512, (half + 1) * 512)
                    for k in range(KH):
                        nc.tensor.matmul(
                            ps_u[:, sl], w1u[:, k], x_sb[:, k, sl],
                            start=(k == 0), stop=(k == KH - 1),
                        )
                s_sb = ap.tile([P, T], f32, tag="silu")
                nc.scalar.activation(
                    s_sb[:], ps_g[:], mybir.ActivationFunctionType.Silu
                )
                nc.vector.tensor_mul(out=g_sb[:, fg, :], in0=s_sb[:], in1=ps_u[:])

            # MM2: psum[h, t] += W2tile^T @ gated^T ; fp32 out
            for ht in range(HT):
                w2t = wp.tile([P, KI, P], mybir.dt.bfloat16, tag="w2")
                nc.sync.dma_start(w2t[:], w2_d[ht])
                ps_o = ps.tile([P, T], f32, tag="ps1")
                for half in range(2):
                    sl = slice(half * 512, (half + 1) * 512)
                    for k in range(KI):
                        nc.tensor.matmul(
                            ps_o[:, sl], w2t[:, k], g_sb[:, k, sl],
                            start=(k == 0), stop=(k == KI - 1),
                        )
                o_sb = op.tile([P, T], f32, tag="o")
                nc.vector.tensor_copy(out=o_sb[:], in_=ps_o[:])
                nc.sync.dma_start(out_d[ht], o_sb[:])

    _orig = type(nc).to_json_bytes
    nc.to_json_bytes = lambda *a, **kw: _split_excess_waits(_orig(nc, *a, **kw))
    return nc


_NC_CACHE = None


def _get_program():
    global _NC_CACHE
    if _NC_CACHE is None:
        _NC_CACHE = _build_program()
    return _NC_CACHE


def prepare_in_maps(hidden_states, gate_up_proj, down_proj):
    hidden_states = np.asarray(hidden_states, dtype=np.float32)
    gate_up_proj = np.asarray(gate_up_proj, dtype=np.float32)
    down_proj = np.asarray(down_proj, dtype=np.float32)

    in_maps = []
    for e in range(E):
        x_e = hidden_states[e * T : (e + 1) * T]                      # [T, H]
        xt = x_e.T.reshape(KH, P, T).transpose(1, 0, 2).astype(BF16)  # [P,KH,T]
        w1 = (
            gate_up_proj[e]
            .reshape(KH, P, 2 * FG, P)
            .transpose(2, 1, 0, 3)
            .astype(BF16)
        )                                                             # [64,P,KH,P]
        w2 = (
            down_proj[e]
            .reshape(KI, P, HT, P)
            .transpose(2, 1, 0, 3)
            .astype(BF16)
        )                                                             # [16,P,KI,P]
        in_maps.append(
            {
                "xt": np.ascontiguousarray(xt),
                "w1": np.ascontiguousarray(w1),
                "w2": np.ascontiguousarray(w2),
            }
        )
    return in_maps


def assemble_out(results):
    out = np.empty((E * T, H), dtype=np.float32)
    for e in range(E):
        r = results[e]["out"]  # [HT, P, T] = out^T tiled
        out[e * T : (e + 1) * T] = r.reshape(H, T).T
    return out


def kernel(hidden_states, gate_up_proj, down_proj):
    in_maps = prepare_in_maps(hidden_states, gate_up_proj, down_proj)
    nc = _get_program()
    res = run_bass_kernel_spmd(nc, in_maps, core_ids=list(range(E)))
    return assemble_out(res.results)



# revision 6
# speedup vs baseline: 76.5243x; 76.5243x over previous
"""Llama4 MoE experts kernel for 8 TRN2 NeuronCores (expert-parallel).

Full-input contract: kernel(**inputs) takes the unsharded fp32 arrays and
returns the full fp32 output. Internally: one expert per core; hidden is
contracted as lhsT=weight-tile (stationary), rhs=x^T (moving), so both
matmul stages produce transposed outputs and no on-chip transpose is
needed. Compute in bf16 (fp32 PSUM accumulate), SiLU on ScalarE, gate*up
on VectorE, output fp32 DMA'd straight from PSUM.

DMA choreography: the fg=0 weight pair is issued first, then x^T in 16
per-k-tile chunks (so the first matmuls start ~4us in instead of waiting
15.8us for one monolithic 4MB x DMA); later weight tiles double-buffer
behind compute. w2 for MM2 is prefetched during MM1 so the PE never
idles at the MM1->MM2 boundary.

Shapes (hardcoded, per spec):
  hidden_states [8192, 2048] f32, gate_up_proj [8, 2048, 8192] f32,
  down_proj [8, 4096, 2048] f32 -> out [8192, 2048] f32.
"""

import ml_dtypes
import numpy as np

import concourse.bass as bass
import concourse.mybir as mybir
import concourse.tile as tile
from concourse.bass_utils import run_bass_kernel_spmd

BF16 = ml_dtypes.bfloat16
P = 128
E = 8          # experts == cores
T = 1024       # tokens per expert
H = 2048       # hidden
I = 4096       # expert dim
KH = H // P    # 16 k-tiles for MM1
KI = I // P    # 32 k-tiles for MM2
FG = 32        # gate f-tiles (up tiles are FG..2*FG-1)
HT = H // P    # 16 output h-tiles


class _TileContext(tile.TileContext):
    """TileContext whose tail drain splits sem waits across instructions.

    The stock _drain_and_barrier attaches every outstanding semaphore wait
    to one Drain instruction; core_v3 codegen only allows one sync wait per
    non-EventSemaphore instruction, so kernels touching >1 semaphore at the
    tail fail with "Too many sync wait commands". Re-emit the extra waits
    as standalone wait_ge instructions ahead of a clean drain.
    """

    def _drain_and_barrier(self, tick_clock, wait_clock):
        import bass_rust as _br

        nc = self.nc
        drain_inst = nc.sync.drain()
        wait_clock.add_sem_waits(
            drain_inst.ins, _br.ScopedClock({None: tick_clock.global_clock})
        )
        si = drain_inst.ins.sync_info
        waits = list(si.on_wait or []) if si is not None else []
        if len(waits) > 1:
            si.on_wait = [waits[0]]
            by_num = {h.num: h for h in self.sems.allocated().values()}
            for w in waits[1:]:
                nc.sync.wait_ge(by_num[w.id], w.wait_value)
            nc.sync.drain()
        nc.all_engine_barrier()
        assert self.sems is not None
        popped = nc._tile_sem_poison_stack.pop()
        assert popped is self._sem_poison
        nc.clear_and_free_semaphores(list(self.sems.allocated().values()))
        nc.all_engine_barrier()


def _split_excess_waits(bir: bytes) -> bytes:
    """Rewrite BIR so no instruction carries more sem waits than this
    walrus accepts (1 per regular instruction, 2 per EventSemaphore).
    Excess waits become standalone EventSemaphore instructions emitted
    just before the over-subscribed instruction on the same engine, which
    is semantically identical (same-engine queue order)."""
    import json

    m = json.loads(bir)
    ctr = 0
    for func in m["functions"]:
        for bb in func["blocks"]:
            out = []
            for ins in bb["instructions"]:
                si = ins.get("sync_info")
                waits = (si or {}).get("on_wait") or []
                cap = 2 if ins.get("opcode") == "EventSemaphore" else 1
                if len(waits) > cap:
                    keep = waits[len(waits) - cap :]
                    excess = waits[: len(waits) - cap]
                    for w in excess:
                        ctr += 1
                        out.append(
                            {
                                "debug": ins.get("debug"),
                                "engine": ins["engine"],
                                "ins": [],
                                "name": f"{ins['name']}-wsplit{ctr}",
                                "opcode": "EventSemaphore",
                                "outs": [],
                                "sync_info": {"on_update": [], "on_wait": [w]},
                            }
                        )
                    si["on_wait"] = keep
                out.append(ins)
            bb["instructions"] = out
    return json.dumps(m).encode()


def _build_program():
    bf16 = mybir.dt.bfloat16
    f32 = mybir.dt.float32

    nc = bass.Bass()
    xt_d = nc.declare_dram_parameter("xt", [P, KH, T], bf16, isOutput=False)
    w1_d = nc.declare_dram_parameter("w1", [2 * FG, P, KH, P], bf16, isOutput=False)
    w2_d = nc.declare_dram_parameter("w2", [HT, P, KI, P], bf16, isOutput=False)
    out_d = nc.declare_dram_parameter("out", [HT, P, T], f32, isOutput=True)

    with _TileContext(nc) as tc:
        with (
            tc.tile_pool(name="xp", bufs=1) as xp,
            tc.tile_pool(name="w1p", bufs=4) as w1p,
            tc.tile_pool(name="w2p", bufs=3) as w2p,
            tc.tile_pool(name="gp", bufs=1) as gp,
            tc.tile_pool(name="ap", bufs=2) as ap,
            tc.tile_pool(name="ps", bufs=3, space="PSUM") as ps,
        ):
            # Feed the head of the pipeline: fg=0 weights in 4-k chunks
            # interleaved with per-k x chunks, ordered so each arrives
            # just before the matmuls that need it. One monolithic x DMA
            # would stall the PE ~15us at the start.
            w1g0 = w1p.tile([P, KH, P], bf16, tag="w1")
            w1u0 = w1p.tile([P, KH, P], bf16, tag="w1")
            x_sb = xp.tile([P, KH, T], bf16)

            def _w1_chunk(c):
                nc.sync.dma_start(w1g0[:, c : c + 4], w1_d[0, :, c : c + 4])
                nc.sync.dma_start(w1u0[:, c : c + 4], w1_d[FG, :, c : c + 4])

            def _x_chunk(ks):
                for k in ks:
                    nc.sync.dma_start(x_sb[:, k], xt_d[:, k])

            _w1_chunk(0)
            _x_chunk([0, 1])
            _w1_chunk(4)
            _x_chunk([2, 3, 4, 5])
            _w1_chunk(8)
            _x_chunk([6, 7, 8, 9])
            _w1_chunk(12)
            _x_chunk(range(10, KH))

            g_sb = gp.tile([P, KI, T], bf16)  # gated^T, cached whole

            # MM1: psum[f, t] += W1tile^T @ x^T ; SiLU-gate -> gated^T (bf16)
            w1g, w1u = w1g0, w1u0
            for fg in range(FG):
                ps_g = ps.tile([P, T], f32, tag="ps1")
                ps_u = ps.tile([P, T], f32, tag="ps1")
                # k-outer so fg=0 tracks the arriving x chunks; two
                # consecutive matmuls share one stationary tile.
                for k in range(KH):
                    st, sp = (k == 0), (k == KH - 1)
                    for half in range(2):
                        sl = slice(half * 512, (half + 1) * 512)
                        nc.tensor.matmul(
                            ps_g[:, sl], w1g[:, k], x_sb[:, k, sl],
                            start=st, stop=sp,
                        )
                    for half in range(2):
                        sl = slice(half * 512, (half + 1) * 512)
                        nc.tensor.matmul(
                            ps_u[:, sl], w1u[:, k], x_sb[:, k, sl],
                            start=st, stop=sp,
                        )
                if fg + 1 < FG:
                    w1g = w1p.tile([P, KH, P], bf16, tag="w1")
                    nc.sync.dma_start(w1g[:], w1_d[fg + 1])
                    w1u = w1p.tile([P, KH, P], bf16, tag="w1")
                    nc.sync.dma_start(w1u[:], w1_d[fg + 1 + FG])
                if fg == 1:
                    # prefetch first MM2 weights well before the boundary
                    w2_first = w2p.tile([P, KI, P], bf16, tag="w2")
                    nc.sync.dma_start(w2_first[:], w2_d[0])
                s_sb = ap.tile([P, T], f32, tag="silu")
                nc.scalar.activation(
                    s_sb[:], ps_g[:], mybir.ActivationFunctionType.Silu
                )
                nc.vector.tensor_mul(out=g_sb[:, fg, :], in0=s_sb[:], in1=ps_u[:])

            # MM2: psum[h, t] += W2tile^T @ gated^T ; fp32 out via SBUF
            # bounce (DMA cannot read PSUM). Copy per-half so half 0's
            # copy+DMA overlap half 1's matmuls.
            w2t = w2_first
            for ht in range(HT):
                ps_o = ps.tile([P, T], f32, tag="ps1")
                o_sb = ap.tile([P, T], f32, tag="o")
                for half in range(2):
                    sl = slice(half * 512, (half + 1) * 512)
                    for k in range(KI):
                        nc.tensor.matmul(
                            ps_o[:, sl], w2t[:, k], g_sb[:, k, sl],
                            start=(k == 0), stop=(k == KI - 1),
                        )
                    nc.vector.tensor_copy(out=o_sb[:, sl], in_=ps_o[:, sl])
                    nc.sync.dma_start(out_d[ht, :, sl], o_sb[:, sl])
                if ht + 1 < HT:
                    w2t = w2p.tile([P, KI, P], bf16, tag="w2")
                    nc.sync.dma_start(w2t[:], w2_d[ht + 1])

    _orig = type(nc).to_json_bytes
    nc.to_json_bytes = lambda *a, **kw: _split_excess_waits(_orig(nc, *a, **kw))
    return nc


_NC_CACHE = None


def _get_program():
    global _NC_CACHE
    if _NC_CACHE is None:
        _NC_CACHE = _build_program()
    return _NC_CACHE


def prepare_in_maps(hidden_states, gate_up_proj, down_proj):
    hidden_states = np.asarray(hidden_states, dtype=np.float32)
    gate_up_proj = np.asarray(gate_up_proj, dtype=np.float32)
    down_proj = np.asarray(down_proj, dtype=np.float32)

    in_maps = []
    for e in range(E):
        x_e = hidden_states[e * T : (e + 1) * T]                      # [T, H]
        xt = x_e.T.reshape(KH, P, T).transpose(1, 0, 2).astype(BF16)  # [P,KH,T]
        w1 = (
            gate_up_proj[e]
            .reshape(KH, P, 2 * FG, P)
            .transpose(2, 1, 0, 3)
            .astype(BF16)
        )                                                             # [64,P,KH,P]
        w2 = (
            down_proj[e]
            .reshape(KI, P, HT, P)
            .transpose(2, 1, 0, 3)
            .astype(BF16)
        )                                                             # [16,P,KI,P]
        in_maps.append(
            {
                "xt": np.ascontiguousarray(xt),
                "w1": np.ascontiguousarray(w1),
                "w2": np.ascontiguousarray(w2),
            }
        )
    return in_maps


def assemble_out(results):
    out = np.empty((E * T, H), dtype=np.float32)
    for e in range(E):
        r = results[e]["out"]  # [HT, P, T] = out^T tiled
        out[e * T : (e + 1) * T] = r.reshape(H, T).T
    return out


def kernel(hidden_states, gate_up_proj, down_proj):
    in_maps = prepare_in_maps(hidden_states, gate_up_proj, down_proj)
    nc = _get_program()
    res = run_bass_kernel_spmd(nc, in_maps, core_ids=list(range(E)))
    return assemble_out(res.results)
